# revision 1
# baseline (speedup 1.0000x reference)
"""Trainium2 Bass kernel for packed-varlen causal attention (16 heads, D=1024).

Strategy: data-parallel over segments across 8 NeuronCores. Each core packs
1-2 segments tile-aligned into a 1536-token buffer. One SPMD program; all
per-core differences are data (packed inputs + 0/1 masks).

Device layout: activations kept feature-major (q^T,k^T = [1024, 1536]) so
attention computes scoresT[k,q] = k^T.T @ q^T directly, softmax denominators
come from an appended ones-column in V during the PV matmul, and PV produces
attn^T feature-major which feeds the output projection with no transposes.
Masks are multiplicative bf16 {0,1} applied after exp (diagonal forced to 1
so padded query columns cannot produce 0/0).
"""
import os
from contextlib import ExitStack

import numpy as np
import ml_dtypes

import concourse.bass as bass
import concourse.tile as tile
from concourse import bacc, mybir
from concourse.bass_utils import run_bass_kernel_spmd

BF16 = ml_dtypes.bfloat16
F32 = np.float32
NCORES = 8
NT = 12            # query tiles of 128 -> 1536 token slots per core
TOK = NT * 128
EMBED, HEADS, HDIM = 1024, 16, 64
DT = mybir.dt

LAST_EXEC_NS = None
LAST_TRACE = None
_CACHE = {}


def _install_ntff_shim():
    """Provide antenv.axon_hooks (missing in this image) so
    run_bass_kernel_spmd(trace=True) can capture NTFF profiles via the
    axon .so, and keep artifacts local instead of uploading."""
    import sys
    import types
    try:
        import antenv.axon_hooks  # noqa: F401
        return
    except ImportError:
        pass
    try:
        from trn_agent_boot.trn_boot import _ntff_profile_via_ctypes
        hook = _ntff_profile_via_ctypes("/opt/axon/libaxon_pjrt.so")
    except Exception:
        hook = None
    mod = types.ModuleType("antenv.axon_hooks")
    mod.get_axon_ntff_profile_hook = lambda: hook
    mod.set_axon_ntff_profile_hook = lambda h: None
    sys.modules["antenv.axon_hooks"] = mod
    import concourse.bass_utils as _bu
    _bu.upload_artifacts = lambda tmpdir: tmpdir


# ---------------------------------------------------------------- planning --

def _build_plan(seq_lens):
    segs = sorted(range(len(seq_lens)), key=lambda i: -int(seq_lens[i]))
    loads = [0.0] * NCORES
    tiles_used = [0] * NCORES
    assign = [[] for _ in range(NCORES)]
    for s in segs:
        L = int(seq_lens[s])
        nt = (L + 127) // 128
        cost = L * 8.4e6 + (L * L) * 2048.0
        placed = False
        for c in sorted(range(NCORES), key=lambda c: loads[c]):
            if tiles_used[c] + nt <= NT:
                assign[c].append(s)
                loads[c] += cost
                tiles_used[c] += nt
                placed = True
                break
        assert placed, "segments do not fit the 8x1536 structure"
    core_chunks = []
    for c in range(NCORES):
        t0, chunks = 0, []
        for s in assign[c]:
            L = int(seq_lens[s])
            chunks.append((s, t0, L))
            t0 += (L + 127) // 128
        core_chunks.append(chunks)
    pairs = set()
    for chunks in core_chunks:
        for (_, t0, L) in chunks:
            nt = (L + 127) // 128
            for a in range(nt):
                for b in range(a + 1):
                    pairs.add((t0 + a, t0 + b))
    structure = []
    for cch in range(3):
        klist = []
        for kj in range(NT):
            qs = [qi for (qi, k2) in pairs
                  if k2 == kj and 4 * cch <= qi < 4 * cch + 4]
            if qs:
                klist.append((kj, min(qs), max(qs) + 1))
        structure.append(klist)
    return core_chunks, structure


# ---------------------------------------------------------- device program --

def _emit_program(structure):
    nc = bacc.Bacc("TRN2", target_bir_lowering=False, debug=False,
                   num_devices=NCORES)
    f32, bf16 = DT.float32, DT.bfloat16
    EXP = mybir.ActivationFunctionType.Exp

    xT_d = nc.dram_tensor("xT", [EMBED, TOK], bf16, kind="ExternalInput").ap()
    cosT_d = nc.dram_tensor("cosT", [128, TOK], f32, kind="ExternalInput").ap()
    sinT_d = nc.dram_tensor("sinT", [128, TOK], f32, kind="ExternalInput").ap()
    maskT_d = nc.dram_tensor("maskT", [TOK, TOK], bf16, kind="ExternalInput").ap()
    wq_d = nc.dram_tensor("wqT", [EMBED, EMBED], bf16, kind="ExternalInput").ap()
    wk_d = nc.dram_tensor("wkT", [EMBED, EMBED], bf16, kind="ExternalInput").ap()
    wv_d = nc.dram_tensor("wvT", [EMBED, EMBED], bf16, kind="ExternalInput").ap()
    wo_d = nc.dram_tensor("woT", [EMBED, EMBED], bf16, kind="ExternalInput").ap()
    qb_d = nc.dram_tensor("qb", [1, EMBED], bf16, kind="ExternalInput").ap()
    vb_d = nc.dram_tensor("vb", [1, EMBED], bf16, kind="ExternalInput").ap()
    ob_d = nc.dram_tensor("ob", [1, EMBED], bf16, kind="ExternalInput").ap()
    sel_d = nc.dram_tensor("sel01", [2, 128], DT.float32r, kind="ExternalInput").ap()
    yT_d = nc.dram_tensor("yT", [EMBED, TOK], f32, kind="ExternalOutput").ap()

    with tile.TileContext(nc) as tc, ExitStack() as ctx:
        singles = ctx.enter_context(tc.tile_pool(name="singles", bufs=1))
        wpool = ctx.enter_context(tc.tile_pool(name="wpool", bufs=2))
        persist = ctx.enter_context(tc.tile_pool(name="persist", bufs=1))
        # PSUM: acc(2) + s0/s1 (2+2, rb shares s0) + pa0/pa1 (1+1) = 8 banks
        accp = ctx.enter_context(tc.tile_pool(name="accp", bufs=2, space="PSUM"))
        spool = ctx.enter_context(tc.tile_pool(name="spool", bufs=2, space="PSUM"))
        papool = ctx.enter_context(tc.tile_pool(name="papool", bufs=1, space="PSUM"))

        # constants / persistent tensors
        qb_sb = singles.tile([1, EMBED], bf16, tag="qb")
        nc.sync.dma_start(out=qb_sb, in_=qb_d)
        vb_sb = singles.tile([1, EMBED], bf16, tag="vb")
        nc.sync.dma_start(out=vb_sb, in_=vb_d)
        ob_sb = singles.tile([1, EMBED], bf16, tag="ob")
        nc.sync.dma_start(out=ob_sb, in_=ob_d)
        sel0_sb = singles.tile([1, 128], DT.float32r, tag="sel0")
        nc.sync.dma_start(out=sel0_sb, in_=sel_d[0:1, :])
        sel1_sb = singles.tile([1, 128], DT.float32r, tag="sel1")
        nc.sync.dma_start(out=sel1_sb, in_=sel_d[1:2, :])
        ones_sb = singles.tile([1, 512], bf16, tag="ones")
        nc.vector.memset(ones_sb, 1.0)

        qr_sb = persist.tile([128, 8, TOK], bf16, tag="qr")
        kr_sb = persist.tile([128, 8, TOK], bf16, tag="kr")
        # v with a ones column appended per head: [tok_tile, head, 65]
        va_sb = persist.tile([128, NT, HEADS, HDIM + 1], bf16, tag="va")
        nc.vector.memset(va_sb[:, :, :, HDIM:HDIM + 1], 1.0)

        def load_w(dram):
            w = wpool.tile([128, 8, EMBED], bf16, tag="w")
            nc.sync.dma_start(out=w, in_=dram.rearrange("(a p) n -> p a n", p=128))
            return w

        # ----------------------------------------------- projections + RoPE
        with tc.tile_pool(name="xpool", bufs=1) as xpool, \
             tc.tile_pool(name="cspool", bufs=1) as cspool, \
             tc.tile_pool(name="rope", bufs=2) as rope:
            x_sb = xpool.tile([128, 8, TOK], bf16, tag="x")
            nc.sync.dma_start(out=x_sb,
                              in_=xT_d.rearrange("(a p) t -> p a t", p=128))
            cos_sb = cspool.tile([128, TOK], f32, tag="cos")
            nc.sync.dma_start(out=cos_sb, in_=cosT_d)
            sin_sb = cspool.tile([128, TOK], f32, tag="sin")
            nc.sync.dma_start(out=sin_sb, in_=sinT_d)

            def rope_proj(w_sb, bias_row, out_sb):
                for m in range(8):
                    for c3 in range(3):
                        t5 = bass.ts(c3, 512)
                        ps = accp.tile([128, 512], f32, tag="acc")
                        for a in range(8):
                            nc.tensor.matmul(ps, w_sb[:, a, bass.ts(m, 128)],
                                             x_sb[:, a, t5], start=(a == 0),
                                             stop=(bias_row is None and a == 7))
                        if bias_row is not None:
                            nc.tensor.matmul(ps, bias_row[:, bass.ts(m, 128)],
                                             ones_sb, start=False, stop=True)
                        qc = rope.tile([128, 512], f32, tag="qc")
                        nc.scalar.copy(qc, ps)
                        sw = rope.tile([128, 512], f32, tag="sw")
                        for half in range(2):
                            b = half * 64
                            nc.sync.dma_start(out=sw[b:b + 32, :],
                                              in_=qc[b + 32:b + 64, :])
                            nc.sync.dma_start(out=sw[b + 32:b + 64, :],
                                              in_=qc[b:b + 32, :])
                        m1 = rope.tile([128, 512], f32, tag="m1")
                        nc.vector.tensor_mul(m1, qc, cos_sb[:, t5])
                        m2 = rope.tile([128, 512], f32, tag="m2")
                        nc.vector.tensor_mul(m2, sw, sin_sb[:, t5])
                        nc.vector.tensor_add(out_sb[:, m, t5], m1, m2)

            wq = load_w(wq_d)
            rope_proj(wq, qb_sb, qr_sb)
            wk = load_w(wk_d)
            rope_proj(wk, None, kr_sb)
            wv = load_w(wv_d)
            for tt in range(NT):
                for n2 in range(2):
                    ps = accp.tile([128, 512], f32, tag="acc")
                    for a in range(8):
                        nc.tensor.matmul(ps, x_sb[:, a, bass.ts(tt, 128)],
                                         wv[:, a, bass.ts(n2, 512)],
                                         start=(a == 0), stop=False)
                    nc.tensor.matmul(ps, ones_sb[:, 0:128],
                                     vb_sb[:, bass.ts(n2, 512)], start=False,
                                     stop=True)
                    nc.scalar.copy(va_sb[:, tt, bass.ts(n2, 8), 0:HDIM],
                                   ps.rearrange("p (h d) -> p h d", d=HDIM))
            wo_sb = load_w(wo_d)

        # ------------------------------------------------------- attention --
        with tc.tile_pool(name="mpool", bufs=1) as mpool, \
             tc.tile_pool(name="epool", bufs=2) as epool, \
             tc.tile_pool(name="rpool", bufs=2) as rpool, \
             tc.tile_pool(name="attnp", bufs=2) as attnp, \
             tc.tile_pool(name="ypool", bufs=2) as ypool:
            for cch in range(3):
                q0 = cch * 512
                klist = structure[cch]
                mts = {}
                for (kj, qlo, qhi) in klist:
                    nq = (qhi - qlo) * 128
                    mt = mpool.tile([128, 512], bf16, tag=f"m{kj % 9}")
                    nc.sync.dma_start(
                        out=mt[:, 0:nq],
                        in_=maskT_d[bass.ts(kj, 128), bass.ds(qlo * 128, nq)])
                    mts[kj] = mt
                attn_sb = attnp.tile([128, 8, 512], bf16, tag="attn")
                for hp in range(8):
                    pas = [papool.tile([HDIM + 1, 512], f32, tag=f"pa{i}",
                                       name=f"pa{i}")
                           for i in range(2)]
                    for ik, (kj, qlo, qhi) in enumerate(klist):
                        nq = (qhi - qlo) * 128
                        qoff = qlo * 128 - q0
                        for i in range(2):
                            h = 2 * hp + i
                            krs = kr_sb[bass.ds((h % 2) * 64, 64), h // 2,
                                        bass.ts(kj, 128)]
                            qrs = qr_sb[bass.ds((h % 2) * 64, 64), h // 2,
                                        bass.ds(qlo * 128, nq)]
                            ps = spool.tile([128, 512], f32, tag=f"s{i}")
                            nc.tensor.matmul(ps[:, 0:nq], krs, qrs,
                                             start=True, stop=True)
                            e = epool.tile([128, 512], bf16, tag=f"e{i}")
                            nc.scalar.activation(e[:, 0:nq], ps[:, 0:nq], EXP,
                                                 scale=0.125)
                            em = epool.tile([128, 512], bf16, tag=f"em{i}")
                            nc.vector.tensor_mul(em[:, 0:nq], e[:, 0:nq],
                                                 mts[kj][:, 0:nq])
                            nc.tensor.matmul(
                                pas[i][:, bass.ds(qoff, nq)],
                                va_sb[:, kj, h, :], em[:, 0:nq],
                                start=(ik == 0), stop=(ik == len(klist) - 1),
                                skip_group_check=True)
                    # normalize: recip of denom row, broadcast via PE, scale
                    rc0 = rpool.tile([1, 512], DT.float32r, tag="rc0")
                    rc1 = rpool.tile([1, 512], DT.float32r, tag="rc1")
                    with nc.allow_low_precision(reason="f32r recip, 10-bit ok"):
                        nc.vector.reciprocal(rc0, pas[0][HDIM:HDIM + 1, :])
                        nc.vector.reciprocal(rc1, pas[1][HDIM:HDIM + 1, :])
                    rb = spool.tile([128, 512], f32, tag="s0")
                    nc.tensor.matmul(rb, sel0_sb, rc0, start=True, stop=False)
                    nc.tensor.matmul(rb, sel1_sb, rc1, start=False, stop=True)
                    rbs = rpool.tile([128, 512], f32, tag="rbs")
                    nc.scalar.copy(rbs, rb)
                    for i in range(2):
                        h = 2 * hp + i
                        nc.vector.tensor_mul(
                            attn_sb[bass.ds((h % 2) * 64, 64), h // 2, :],
                            pas[i][0:HDIM, :], rbs[bass.ds(i * 64, 64), :])
                # -------------------------------------------- out-projection
                for m in range(8):
                    py = accp.tile([128, 512], f32, tag="acc")
                    for r in range(8):
                        nc.tensor.matmul(py, wo_sb[:, r, bass.ts(m, 128)],
                                         attn_sb[:, r, :], start=(r == 0),
                                         stop=False)
                    nc.tensor.matmul(py, ob_sb[:, bass.ts(m, 128)], ones_sb,
                                     start=False, stop=True)
                    ys = ypool.tile([128, 512], f32, tag="ys")
                    nc.scalar.copy(ys, py)
                    nc.sync.dma_start(
                        out=yT_d[bass.ts(m, 128), bass.ts(cch, 512)], in_=ys)
    nc.compile()
    return nc


# ------------------------------------------------------------- host driver --

def _host_prep(hidden, cos, sin, seq_lens, core_chunks):
    starts = np.concatenate([[0], np.cumsum(seq_lens)]).astype(np.int64)
    per_core = []
    sgn = np.concatenate([-np.ones(32, F32), np.ones(32, F32)])
    for c in range(NCORES):
        tokmap = np.full(TOK, -1, np.int64)
        segid = np.full(TOK, -1, np.int64)
        pos = np.zeros(TOK, np.int64)
        for (s, t0, L) in core_chunks[c]:
            sl = slice(t0 * 128, t0 * 128 + L)
            tokmap[sl] = np.arange(starts[s], starts[s] + L)
            segid[sl] = s
            pos[sl] = np.arange(L)
        real = tokmap >= 0
        x = np.zeros((TOK, EMBED), F32)
        x[real] = hidden[tokmap[real]]
        cs = np.zeros((TOK, HDIM), F32)
        sn = np.zeros((TOK, HDIM), F32)
        cs[real] = cos[tokmap[real]]
        sn[real] = sin[tokmap[real]]
        cosT = np.tile(np.ascontiguousarray(cs.T), (2, 1)).astype(F32)
        sinT = np.tile(np.ascontiguousarray(sn.T) * sgn[:, None],
                       (2, 1)).astype(F32)
        same = (segid[:, None] == segid[None, :]) & (segid[:, None] >= 0)
        causal = pos[:, None] <= pos[None, :]
        m01 = (same & causal) | np.eye(TOK, dtype=bool)
        maskT = m01.astype(BF16)
        per_core.append(dict(tokmap=tokmap,
                             xT=np.ascontiguousarray(x.T).astype(BF16),
                             cosT=cosT, sinT=sinT, maskT=maskT))
    return per_core


def kernel(hidden_states, cos, sin, q_w, q_b, k_w, v_w, v_b, out_w, out_b,
           seq_len, max_seqlen):
    global LAST_EXEC_NS
    hidden = np.asarray(hidden_states, F32)
    cos = np.asarray(cos, F32)
    sin = np.asarray(sin, F32)
    seq_lens = [int(v) for v in np.asarray(seq_len)]

    core_chunks, structure = _build_plan(seq_lens)
    key = tuple(tuple(map(tuple, s)) for s in structure)
    if key not in _CACHE:
        _CACHE[key] = _emit_program(structure)
    nc = _CACHE[key]

    per_core = _host_prep(hidden, cos, sin, seq_lens, core_chunks)
    sel01 = np.zeros((2, 128), F32)
    sel01[0, 0:64] = 1.0
    sel01[1, 64:128] = 1.0
    shared = {
        "wqT": np.ascontiguousarray(np.asarray(q_w, F32).T).astype(BF16),
        "wkT": np.ascontiguousarray(np.asarray(k_w, F32).T).astype(BF16),
        "wvT": np.ascontiguousarray(np.asarray(v_w, F32).T).astype(BF16),
        "woT": np.ascontiguousarray(np.asarray(out_w, F32).T).astype(BF16),
        "qb": np.asarray(q_b, F32).reshape(1, EMBED).astype(BF16),
        "vb": np.asarray(v_b, F32).reshape(1, EMBED).astype(BF16),
        "ob": np.asarray(out_b, F32).reshape(1, EMBED).astype(BF16),
        "sel01": sel01,
    }
    in_maps = []
    for c in range(NCORES):
        pc = per_core[c]
        in_maps.append({**shared, "xT": pc["xT"], "cosT": pc["cosT"],
                        "sinT": pc["sinT"], "maskT": pc["maskT"]})

    trace = os.environ.get("BASS_KERNEL_TRACE", "0") == "1"
    if trace:
        _install_ntff_shim()
    import time as _time
    _t0 = _time.time()
    res = run_bass_kernel_spmd(nc, in_maps, core_ids=list(range(NCORES)),
                               trace=trace)
    LAST_EXEC_NS = res.exec_time_ns
    globals()["LAST_TRACE"] = res.instructions_and_trace
    globals()["LAST_RUN_WALL_S"] = _time.time() - _t0

    T = hidden.shape[0]
    out = np.zeros((T, EMBED), F32)
    for c in range(NCORES):
        tokmap = per_core[c]["tokmap"]
        real = tokmap >= 0
        yT = np.asarray(res.results[c]["yT"], F32)
        out[tokmap[real]] = yT.T[real]
    return out

